# revision 10
# baseline (speedup 1.0000x reference)
"""Trainium2 Bass kernel: BiDAF-style context-query attention (nn_CQattn).

Reference (per batch b):
    S    = (C@w1)[:,None] + (Q@w2)[None,:] + (C*w3) @ Q.T        # [N, M]
    S1   = softmax_m(S + NEG*Qmask[None,:])                      # row softmax
    S2   = softmax_n(S + NEG*Cmask[:,None])                      # col softmax
    A    = S1 @ Q                                                # [N, D]
    Bout = S1 @ (S2.T @ C)                                       # [N, D]

Device algorithm (one exp'd matrix serves both softmaxes):
  X[n,m] = exp(dot3[n,m] + q2[m] + c1[n]) = exp(S[n,m]), computed as ONE
  matmul chain: q2 folds into the stationary operand via
  CW = C*w3 + w2  (since dot3 + q2 = sum_d CW[n,d]*Q[m,d]), and c1 = C@w1
  is a host-precomputed per-partition ACT bias.  Masks never touch X:
    - row softmax (A path): any per-n factor cancels; Qmask folds into the
      rhs operands (host-zeroed Qz rows, 0/1 zq vector replacing `ones`,
      and a zq-zeroed T).
    - col softmax (T path): any per-m factor cancels; Cmask folds into the
      rhs (host-zeroed Cz rows, 0/1 zC sum vector).
  A = (X.T' @ Qz) / (X.T' @ zq),  T = (X' @ Cz)/(X' @ zC) * zq,
  Bout = (X.T' @ T) / (X.T' @ zq);  X.T via PE transpose (bf16, 1c/row).

Mask compaction: the host permutes n and m (unmasked first) per batch, so
only MCAP=384 of 512 m-columns and NCAP=1280 of 2048 n-rows participate in
the masked paths (actual unmasked counts are ~236-286 m / ~981-1065 n; caps
sit ~9 sigma above a Bernoulli(0.5) tail so any regenerated inputs fit).
Outputs come back n-permuted; the host inverse-permutes.  All operands are
bf16 (tolerance 2e-2; measured rel err ~3e-3); PSUM accumulation is fp32.
The host also ships pre-transposed layouts (CW.T, Q.T) so the device does
ZERO input transposes; only X needs an on-device PE transpose.

Sharding: data-parallel over batch: 32 batches / 8 cores = 4 per core.
Self-contained: shapes hardcoded; no sibling imports.

Toolchain note: the walrus build in this container accepts at most one
sem-wait per instruction, while Tile's scheduler attaches several; the
_patch_tile_drain_wait_split hook below splits excess waits onto
same-engine NOPs (required for ANY Tile kernel to compile here).
"""

import numpy as np

B, N, M, D = 32, 2048, 512, 512
NCORES = 8
BPC = B // NCORES  # batches per core

NT = N // 128   # 16 n-tiles (full)
DT = D // 128   # 4 d-tiles
MCAP = 384      # m capacity after compaction (3 tiles)
MT = MCAP // 128
NCAP = 1152     # n capacity for the col-softmax (T) contraction
NTT = NCAP // 128  # 9

# Transpose X via the DMA crossbar (16x128 xbar tiles, bf16) instead of PE
# matmul-transposes: frees ~6.1k PE cycles/batch; issue alternates between
# the SP and ACT HWDGE queues to balance sequencer load.
TR_VIA_DMA = True


def _patch_tile_drain_wait_split():
    """The stock Tile kernel-tail drain carries one sem-wait per still-pending
    proc on a single InstDrain; the walrus build in this container rejects >1
    sync wait per instruction ("Too many sync wait commands").  Split the
    excess waits onto dedicated sync-engine NOPs emitted right after the
    drain (they still precede the all-engine barrier, preserving the
    everything-done-before-teardown guarantee)."""
    import concourse.mybir as mybir
    import concourse.tile as tile

    if getattr(tile.TileContext, "_drain_wait_split_patched", False):
        return

    orig_add = tile.TileContext._add_instruction

    def _add_instruction(self, inst):
        si = inst.sync_info
        waits = list(si.on_wait) if si and si.on_wait else []
        if len(waits) > 1 and inst.engine != mybir.EngineType.Unassigned:
            for w in waits[:-1]:
                nop = mybir.InstNoOp(
                    name=self.nc.get_next_instruction_name(), ins=[], outs=[]
                )
                nop.engine = inst.engine
                nop.sync_info = mybir.SyncInfo(on_wait=[w], on_update=[])
                orig_add(self, nop)
            inst.sync_info = mybir.SyncInfo(
                on_wait=[waits[-1]],
                on_update=list(si.on_update) if si.on_update else [],
            )
        orig_add(self, inst)

    tile.TileContext._add_instruction = _add_instruction

    def _drain_and_barrier(self, tick_clock, wait_clock):
        nc = self.nc
        drain_inst = nc.sync.drain()
        wait_clock.add_sem_waits(
            drain_inst.ins, tile.ScopedClock({None: tick_clock.global_clock})
        )
        si = drain_inst.ins.sync_info
        waits = list(si.on_wait) if si and si.on_wait else []
        if len(waits) > 1:
            drain_inst.ins.sync_info = mybir.SyncInfo(
                on_wait=[waits[0]],
                on_update=list(si.on_update) if si and si.on_update else [],
            )
            for w in waits[1:]:
                nop = nc.sync.nop(nofuse=True, hint="drain_wait_split")
                nop.ins.sync_info = mybir.SyncInfo(on_wait=[w], on_update=[])

        nc.all_engine_barrier()
        assert self.sems is not None
        popped = nc._tile_sem_poison_stack.pop()
        assert popped is self._sem_poison
        nc.clear_and_free_semaphores(list(self.sems.allocated().values()))
        nc.all_engine_barrier()

    tile.TileContext._drain_and_barrier = _drain_and_barrier
    tile.TileContext._drain_wait_split_patched = True


def build_nc(n_reps=1):
    import concourse.bass as bass
    import concourse.mybir as mybir
    import concourse.tile as tile

    _patch_tile_drain_wait_split()

    f32 = mybir.dt.float32
    bf16 = mybir.dt.bfloat16
    AF = mybir.ActivationFunctionType

    nc = bass.Bass()
    # All inputs shipped in exact SBUF layout: [128 partitions, ...].
    ctw_d = nc.dram_tensor("CTW", [BPC, 128, DT, N], bf16, kind="ExternalInput")
    qt_d = nc.dram_tensor("QT", [BPC, 128, DT, MCAP], bf16, kind="ExternalInput")
    cz_d = nc.dram_tensor("CZ", [BPC, 128, NTT, D], bf16, kind="ExternalInput")
    qz_d = nc.dram_tensor("QZ", [BPC, 128, MT, D], bf16, kind="ExternalInput")
    c1b_d = nc.dram_tensor("c1b", [128, BPC, NT], f32, kind="ExternalInput")
    zcb_d = nc.dram_tensor("zCb", [128, BPC, NTT], bf16, kind="ExternalInput")
    zqb_d = nc.dram_tensor("zqb", [128, BPC, MT], bf16, kind="ExternalInput")
    zqf_d = nc.dram_tensor("zqf", [128, BPC, MT], f32, kind="ExternalInput")
    id_d = nc.dram_tensor("ident", [128, 128], bf16, kind="ExternalInput")
    A_d = nc.dram_tensor("A", [BPC, 128, NT, D], bf16, kind="ExternalOutput")
    Bo_d = nc.dram_tensor("Bout", [BPC, 128, NT, D], bf16, kind="ExternalOutput")

    mm = None  # set inside context

    with tile.TileContext(nc) as tc:
        with (
            tc.tile_pool(name="const", bufs=1) as constp,
            tc.tile_pool(name="ctw", bufs=2) as ctwp,
            tc.tile_pool(name="qtp", bufs=2) as qtp,
            tc.tile_pool(name="czp", bufs=2) as czp,
            tc.tile_pool(name="qzp", bufs=2) as qzp,
            tc.tile_pool(name="xp", bufs=NT + 4) as xp,
            tc.tile_pool(name="xtp", bufs=2) as xtp,
            tc.tile_pool(name="tzp", bufs=2) as tzp,
            tc.tile_pool(name="smallp", bufs=24) as smallp,
            tc.tile_pool(name="astp", bufs=2) as astp,
            tc.tile_pool(name="bstp", bufs=2) as bstp,
            tc.tile_pool(name="ps_s", bufs=3, space="PSUM") as ps_s,
            tc.tile_pool(name="ps_b", bufs=2, space="PSUM") as ps_b,
            tc.tile_pool(name="ps_ab", bufs=2, space="PSUM") as ps_ab,
            tc.tile_pool(name="pss", bufs=1, space="PSUM") as pss,
        ):
            ident = constp.tile([128, 128], bf16, name="ident")
            nc.sync.dma_start(ident[:], id_d[:])
            c1b = constp.tile([128, BPC, NT], f32, name="c1b")
            nc.sync.dma_start(c1b[:], c1b_d[:])
            zcb = constp.tile([128, BPC, NTT], bf16, name="zcb")
            nc.sync.dma_start(zcb[:], zcb_d[:])
            zqb = constp.tile([128, BPC, MT], bf16, name="zqb")
            nc.sync.dma_start(zqb[:], zqb_d[:])
            zqf = constp.tile([128, BPC, MT], f32, name="zqf")
            nc.sync.dma_start(zqf[:], zqf_d[:])

            def load_batch(b):
                ctw = ctwp.tile([128, DT, N], bf16, name="ctw", tag="ctw")
                nc.sync.dma_start(ctw[:], ctw_d[b])
                qt = qtp.tile([128, DT, MCAP], bf16, name="qt", tag="qt")
                nc.sync.dma_start(qt[:], qt_d[b])
                cz = czp.tile([128, NTT, D], bf16, name="cz", tag="cz")
                nc.sync.dma_start(cz[:], cz_d[b])
                qz = qzp.tile([128, MT, D], bf16, name="qz", tag="qz")
                nc.sync.dma_start(qz[:], qz_d[b])
                return ctw, qt, cz, qz

            batches = [b for _ in range(n_reps) for b in range(BPC)]
            cur = load_batch(batches[0])
            for i, b in enumerate(batches):
                ctw, qt, cz, qz = cur

                # ---- S phase: X[t] = exp(dot3 + q2 + c1)   [128 n, 384 m]
                x_tiles = []
                for t in range(NT):
                    ps = ps_s.tile([128, MCAP], f32, name="ps_s", tag="ps_s")
                    for j in range(DT):
                        nc.tensor.matmul(
                            ps[:],
                            ctw[:, j, t * 128 : (t + 1) * 128],
                            qt[:, j, :],
                            start=(j == 0),
                            stop=(j == DT - 1),
                        )
                    xt_ = xp.tile([128, MCAP], bf16, name="X", tag="X")
                    nc.scalar.activation(
                        xt_[:], ps[:], AF.Exp, bias=c1b[:, b, t : t + 1]
                    )
                    x_tiles.append(xt_)

                # ---- transpose phase: XT[u] = X.T   [128 m, 2048 n]
                xtt = xtp.tile([128, MT, N], bf16, name="XT", tag="XT")
                if TR_VIA_DMA:
                    # logical row m of the transposed block lands at
                    # xtt[m % 128, m // 128, t*128 + c]  (verified in CoreSim)
                    for t in range(NT):
                        nc.sync.dma_start_transpose(
                            out=xtt[:, :, t * 128 : (t + 1) * 128],
                            in_=x_tiles[t][:],
                        )
                else:
                    for u in range(MT):
                        for nq in range(NT // 4):
                            ps = ps_b.tile([128, 512], bf16, name="ps_tr", tag="ps_b")
                            for s in range(4):
                                t = nq * 4 + s
                                nc.tensor.transpose(
                                    ps[:, s * 128 : (s + 1) * 128],
                                    x_tiles[t][:, u * 128 : (u + 1) * 128],
                                    ident[:],
                                )
                            nc.vector.tensor_copy(
                                xtt[:, u, nq * 512 : (nq + 1) * 512], ps[:]
                            )

                if i + 1 < len(batches):
                    cur = load_batch(batches[i + 1])

                # ---- T phase: Tz[u] = zq * (X' @ Cz) / (X' @ zC)
                # All [128,1] sum accumulators pack as columns of ONE shared
                # PSUM tile (independent accumulation groups, one bank).
                psm = pss.tile([128, 32], f32, name="ps_sums", tag="pss")
                tz = tzp.tile([128, MT, D], bf16, name="Tz", tag="Tz")
                for u in range(MT):
                    pst = ps_b.tile([128, 512], f32, name="ps_T", tag="ps_b")
                    pcs = psm[:, 16 + u : 17 + u]
                    for t in range(NTT):
                        lhsT = x_tiles[t][:, u * 128 : (u + 1) * 128]
                        nc.tensor.matmul(
                            pst[:], lhsT, cz[:, t, :],
                            start=(t == 0), stop=(t == NTT - 1),
                        )
                        nc.tensor.matmul(
                            pcs, lhsT, zcb[:, b, t : t + 1],
                            start=(t == 0), stop=(t == NTT - 1),
                        )
                    r2 = smallp.tile([128, 1], f32, name="r2", tag="small")
                    nc.vector.reciprocal(r2[:], pcs)
                    r2z = smallp.tile([128, 1], f32, name="r2z", tag="small")
                    nc.vector.tensor_scalar_mul(r2z[:], r2[:], zqf[:, b, u : u + 1])
                    nc.scalar.activation(tz[:, u, :], pst[:], AF.Copy, scale=r2z[:])

                # ---- A/B phase: per n-tile, contract m over MT tiles
                for g in range(NT // 4):
                    ast = astp.tile([128, 4, D], bf16, name="Ast", tag="Ast")
                    bst = bstp.tile([128, 4, D], bf16, name="Bst", tag="Bst")
                    for s in range(4):
                        t = g * 4 + s
                        psa = ps_ab.tile([128, 512], f32, name="ps_A", tag="ps_ab")
                        psb2 = ps_ab.tile([128, 512], f32, name="ps_B", tag="ps_ab")
                        psr = psm[:, t : t + 1]
                        for u in range(MT):
                            lhsT = xtt[:, u, t * 128 : (t + 1) * 128]
                            nc.tensor.matmul(
                                psa[:], lhsT, qz[:, u, :],
                                start=(u == 0), stop=(u == MT - 1),
                            )
                            nc.tensor.matmul(
                                psb2[:], lhsT, tz[:, u, :],
                                start=(u == 0), stop=(u == MT - 1),
                            )
                            nc.tensor.matmul(
                                psr, lhsT, zqb[:, b, u : u + 1],
                                start=(u == 0), stop=(u == MT - 1),
                            )
                        r1 = smallp.tile([128, 1], f32, name="r1", tag="small")
                        nc.vector.reciprocal(r1[:], psr)
                        nc.vector.tensor_scalar_mul(ast[:, s, :], psa[:], r1[:])
                        nc.scalar.activation(
                            bst[:, s, :], psb2[:], AF.Copy, scale=r1[:]
                        )
                    nc.sync.dma_start(A_d[b, :, g * 4 : (g + 1) * 4, :], ast[:])
                    nc.sync.dma_start(Bo_d[b, :, g * 4 : (g + 1) * 4, :], bst[:])

    return nc


_NC = None


def _get_nc():
    global _NC
    if _NC is None:
        _NC = build_nc()
        _NC.finalize()
    return _NC


def _make_in_maps(C, Q, Cmask, Qmask, w):
    import ml_dtypes

    bf16 = ml_dtypes.bfloat16
    C = np.asarray(C, dtype=np.float32)
    Q = np.asarray(Q, dtype=np.float32)
    Cmask = np.asarray(Cmask)
    Qmask = np.asarray(Qmask)
    w = np.asarray(w, dtype=np.float32)
    w1, w2, w3 = w[:D], w[D : 2 * D], w[2 * D :]

    ident = np.eye(128, dtype=bf16)

    # Per-batch host prep: permute unmasked-first, fold weights/masks, cast.
    CTW = np.empty((B, 128, DT, N), dtype=bf16)
    QT = np.empty((B, 128, DT, MCAP), dtype=bf16)
    CZ = np.empty((B, 128, NTT, D), dtype=bf16)
    QZ = np.empty((B, 128, MT, D), dtype=bf16)
    c1b = np.empty((128, B, NT), dtype=np.float32)
    zCb = np.empty((128, B, NTT), dtype=bf16)
    zqb = np.empty((128, B, MT), dtype=bf16)
    norders = np.empty((B, N), dtype=np.int64)
    for b in range(B):
        no = np.argsort(Cmask[b], kind="stable")
        mo = np.argsort(Qmask[b], kind="stable")
        assert (Cmask[b] == 0).sum() <= NCAP, "NCAP exceeded"
        assert (Qmask[b] == 0).sum() <= MCAP, "MCAP exceeded"
        norders[b] = no
        Cp = C[b][no]                      # [N, D]
        Qp = Q[b][mo[:MCAP]]               # [MCAP, D]
        zq = (1 - Qmask[b][mo[:MCAP]]).astype(np.float32)
        zC = (1 - Cmask[b][no[:NCAP]]).astype(np.float32)
        CW = Cp * w3[None, :] + w2[None, :]
        # transposed layouts, partition = d % 128
        CTW[b] = CW.T.reshape(DT, 128, N).transpose(1, 0, 2)
        QT[b] = Qp.T.reshape(DT, 128, MCAP).transpose(1, 0, 2)
        CZ[b] = (Cp[:NCAP] * zC[:, None]).reshape(NTT, 128, D).transpose(1, 0, 2)
        QZ[b] = (Qp * zq[:, None]).reshape(MT, 128, D).transpose(1, 0, 2)
        c1b[:, b, :] = (Cp @ w1).reshape(NT, 128).T
        zCb[:, b, :] = zC.reshape(NTT, 128).T
        zqb[:, b, :] = zq.reshape(MT, 128).T

    in_maps = []
    for c in range(NCORES):
        bs = slice(c * BPC, (c + 1) * BPC)
        in_maps.append(
            {
                "CTW": np.ascontiguousarray(CTW[bs]),
                "QT": np.ascontiguousarray(QT[bs]),
                "CZ": np.ascontiguousarray(CZ[bs]),
                "QZ": np.ascontiguousarray(QZ[bs]),
                "c1b": np.ascontiguousarray(c1b[:, bs, :]),
                "zCb": np.ascontiguousarray(zCb[:, bs, :]),
                "zqb": np.ascontiguousarray(zqb[:, bs, :]),
                "zqf": np.ascontiguousarray(zqb[:, bs, :]).astype(np.float32),
                "ident": ident,
            }
        )
    return in_maps, norders


def run_spmd(C, Q, Cmask, Qmask, w, trace=False):
    """Returns ((A, Bout), BassKernelResults)."""
    from concourse.bass_utils import run_bass_kernel_spmd

    nc = _get_nc()
    in_maps, norders = _make_in_maps(C, Q, Cmask, Qmask, w)
    res = run_bass_kernel_spmd(nc, in_maps, list(range(NCORES)), trace=trace)
    # device A/B: [BPC, 128, NT, D] bf16 with n = t*128 + p, n-permuted
    A = np.empty((B, N, D), dtype=np.float32)
    Bout = np.empty((B, N, D), dtype=np.float32)
    for c in range(NCORES):
        for i in range(BPC):
            b = c * BPC + i
            no = norders[b]
            a_dev = np.asarray(res.results[c]["A"][i], dtype=np.float32)
            b_dev = np.asarray(res.results[c]["Bout"][i], dtype=np.float32)
            A[b][no] = a_dev.transpose(1, 0, 2).reshape(N, D)
            Bout[b][no] = b_dev.transpose(1, 0, 2).reshape(N, D)
    return (A, Bout), res


def kernel(C, Q, Cmask, Qmask, w):
    (A, Bout), _ = run_spmd(C, Q, Cmask, Qmask, w, trace=False)
    return (A, Bout)
